# revision 4
# baseline (speedup 1.0000x reference)
"""Trainium2 Bass kernel (v3) for nn_Attention (dense transformer block), 8 cores.

Sharding: data-parallel over (batch, L/2) -> 8 shards; each core recomputes
K/V for its batch's full 2048 keys (key order rotated so "own half first" is
a single SPMD program) and runs attention + output projection + layernorm
for its own 1024 tokens. No collectives.

Device design:
  - Stage 1 (QKV projections) runs fp8e4m3 DoubleRow matmuls (x at unit
    scale, weights x16, host-quantized pair-concat layout), f32 psum, silu
    on ACT.  K^T/Q^T are written back as fp8 (unit scale), so the score
    matmuls also run fp8 DoubleRow at 0.5 cycles/row: the pair dim of kt/qt
    is a broadcast (stride-0) view, which doubles the contraction; the exp
    scale absorbs the factor 2.  Zeroed dead halves of qtA/qtB mask the
    other head's dims in the shared kt tile.
  - Phase structure keeps the ACT activation-table clean (3 loads/exec vs 26
    in v1): head-pair 0 runs attention with all-DVE Schraudolph exp while
    ACT still owns the silu table; the V projection and the remaining K/Q
    projection blocks are emitted just-in-time inside pair 0's key-chunk
    loop, so every silu lands before the first ACT exp and PE/DVE stay fed.
    Pairs 1..7 alternate exact ACT exp / DVE Schraudolph per score tile.
  - att@v accumulates [65, L/2] psum per head with 512-free matmuls; the
    65th row (v65 all-ones column) accumulates the softmax denominator.
    attn_tail stages unnormalized outputs in bf16 and denominator rows in
    dstk; norm_half batches: DMA-gather -> one reciprocal per 8 heads ->
    selector-matmul broadcast (selector value 4.0 folds the fp8 scale) ->
    normalize into fp8 nvu2 (pair-concat layout for stage-3 DoubleRow).
  - Stage 3 (out-proj + swish + residual + layernorm) uses fp8 DoubleRow
    matmuls (nvu2 x4, W_out x16), silu/Square-with-accumulator on ACT
    (free row sums), residual sums precomputed while DVE idles at start,
    and rsqrt(var+eps) via the i32 bit trick + 2 Newton steps on DVE (no
    sqrt table load).  Final scale on GPSIMD.
"""
import numpy as np
import ml_dtypes

from concourse import bacc, tile, mybir
from concourse.bass_utils import run_bass_kernel_spmd
from concourse import masks

F32 = mybir.dt.float32
BF16 = mybir.dt.bfloat16
FP8 = mybir.dt.float8e4
I16 = mybir.dt.int16
I32 = mybir.dt.int32
AF = mybir.ActivationFunctionType
ALU = mybir.AluOpType
BF = ml_dtypes.bfloat16
E4 = ml_dtypes.float8_e4m3

B, L, D, H, HD = 4, 2048, 1024, 16, 64
EPS = 1e-5
N_CORES = 8
LH = L // 2          # own tokens per core (1024)
NKC = L // 128       # key chunks (16)
NQT = LH // 128      # own-token q tiles (8)
NC8 = D // 128       # 128-feature chunks of D (8)

INV16 = 1.0 / 16.0   # undo x(=1) * W(x16) fp8 scaling in stage-1 psums
# exp scale: scores in psum are 2 * (q.k) (stride-0 pair double count)
EXP_SCL = 0.5
SCH_A = 128.0 / np.log(2.0) * EXP_SCL   # Schraudolph scale folded
SCH_B = 127.0 * 128.0 - 9.3

# steady-state exp split alternates ACT/DVE; pair 0 is all-DVE (it runs
# inside the silu window where ACT is busy and cannot hold the exp table)
EXP_PAT = ["D", "A"]


def build_nc(reps=1):
    nc = bacc.Bacc("TRN2", target_bir_lowering=False, debug=False,
                   num_devices=N_CORES, enable_partition_id=False)

    # xtw = [xt (8*128) | wq (4*128) | wk (4*128) | wv (4*128) | xr | wo]
    # xr/wo are bf16 shipped as raw bytes, bitcast on the DMA.
    XW0 = 20 * 128  # 2560
    xtw_ext = nc.dram_tensor("xtw", [XW0 + LH + D, 2 * D], FP8,
                             kind="ExternalInput")
    out_ext = nc.dram_tensor("out", [LH, D], F32, kind="ExternalOutput")

    with tile.TileContext(nc) as tc:
        with (
            tc.tile_pool(name="per", bufs=1) as per,      # persistent tiles
            tc.tile_pool(name="att", bufs=6) as attp,     # att bf16 stream tiles
            tc.tile_pool(name="s3", bufs=3) as s3p,       # stage-3 tiles
        ):
            # ---------------- persistent tiles + input DMAs ----------------
            xt = [per.tile([128, 2 * L], FP8, tag=f"xt{i}", name=f"xt{i}")
                  for i in range(4)]
            wq = [per.tile([128, 2 * D], FP8, tag=f"wq{i}", name=f"wq{i}")
                  for i in range(4)]
            wk = [per.tile([128, 2 * D], FP8, tag=f"wk{i}", name=f"wk{i}")
                  for i in range(4)]
            wv = [per.tile([128, 2 * D], FP8, tag=f"wv{i}", name=f"wv{i}")
                  for i in range(4)]
            wo = [per.tile([128, 2 * D], FP8, tag=f"wo{i}", name=f"wo{i}")
                  for i in range(4)]
            xr8 = [per.tile([128, D], BF16, tag=f"xr{i}", name=f"xr{i}")
                   for i in range(NQT)]
            v65 = [per.tile([128, H * 65], BF16, tag=f"v65_{i}", name=f"v65_{i}")
                   for i in range(NKC)]
            kt8 = [per.tile([128, L], FP8, tag=f"kt{i}", name=f"kt{i}")
                   for i in range(NC8)]
            qtA = [per.tile([128, LH], FP8, tag=f"qtA{i}", name=f"qtA{i}")
                   for i in range(NC8)]
            qtB = [per.tile([128, LH], FP8, tag=f"qtB{i}", name=f"qtB{i}")
                   for i in range(NC8)]
            nvu2 = [per.tile([128, 2 * LH], FP8, tag=f"nvu{i}", name=f"nvu{i}")
                    for i in range(NC8 // 2)]
            sel = per.tile([64, 4 * 128], BF16, tag="sel")
            dstk = per.tile([128, 4 * LH], BF16, tag="dstk")  # denom staging
            dsb = per.tile([64, LH], BF16, tag="dsb")

            WQ0, WK0, WV0 = 8 * 128, 12 * 128, 16 * 128

            def input_dmas():
                # interleaved so proj_kq(0) j-loop chases the DMA stream
                for i in range(4):
                    nc.sync.dma_start(
                        xt[i][:, 0:2 * D],
                        xtw_ext[(2 * i) * 128:(2 * i + 1) * 128, :])
                    nc.sync.dma_start(
                        xt[i][:, 2 * D:4 * D],
                        xtw_ext[(2 * i + 1) * 128:(2 * i + 2) * 128, :])
                    nc.sync.dma_start(
                        wk[i][:], xtw_ext[WK0 + i * 128:WK0 + (i + 1) * 128, :])
                    nc.sync.dma_start(
                        wv[i][:], xtw_ext[WV0 + i * 128:WV0 + (i + 1) * 128, :])
                    nc.sync.dma_start(
                        wq[i][:], xtw_ext[WQ0 + i * 128:WQ0 + (i + 1) * 128, :])
                nc.sync.dma_start(
                    sel[:], xtw_ext[SEL0:SEL0 + 64, :].bitcast(BF16)[:, 0:512])
                # stage-3 operands (DMA idles mid-kernel; these land early)
                for i in range(NQT):
                    nc.sync.dma_start(
                        xr8[i][:],
                        xtw_ext[XW0 + i * 128:XW0 + (i + 1) * 128, :].bitcast(BF16))
                for i in range(4):
                    nc.sync.dma_start(
                        wo[i][:],
                        xtw_ext[XW0 + LH + i * 128:XW0 + LH + (i + 1) * 128, :])

            def setup_once():
                # dead halves of q tiles must be zero: they mask the other
                # head's dims in the shared kt tile during score matmuls
                for i in range(NC8):
                    nc.gpsimd.memset(qtA[i][64:128, :], 0.0)
                    nc.gpsimd.memset(qtB[i][0:64, :], 0.0)
                for t in range(NKC):
                    ones_cols = v65[t][:].rearrange(
                        "p (h e) -> p h e", e=65)[:, :, 64:65]
                    nc.gpsimd.memset(ones_cols, 1.0)

            # pair views for stage-1 DoubleRow operands
            xtv = [t[:].rearrange("p (k n) -> p k n", k=2) for t in xt]
            wqv = [t[:].rearrange("p (k n) -> p k n", k=2) for t in wq]
            wkv = [t[:].rearrange("p (k n) -> p k n", k=2) for t in wk]
            wvv = [t[:].rearrange("p (k n) -> p k n", k=2) for t in wv]
            # broadcast (stride-0) pair views for fp8-DR score matmuls
            ktb = [t[:].rearrange("p (o n) -> p o n", o=1).broadcast_to(
                [128, 2, L]) for t in kt8]
            qtAb = [t[:].rearrange("p (o n) -> p o n", o=1).broadcast_to(
                [128, 2, LH]) for t in qtA]
            qtBb = [t[:].rearrange("p (o n) -> p o n", o=1).broadcast_to(
                [128, 2, LH]) for t in qtB]
            wov = [t[:].rearrange("p (k n) -> p k n", k=2) for t in wo]
            nvu2v = [t[:].rearrange("p (k n) -> p k n", k=2) for t in nvu2]
            DR = mybir.MatmulPerfMode.DoubleRow

            def proj_v(t, psp):
                """V projection for key chunk t (token-major, silu, bf16)."""
                for g in range(2):
                    ps = psp.tile([128, 512], F32, tag="sc", name=f"psv{t}{g}")
                    for j in range(4):
                        nc.tensor.matmul(
                            ps[:],
                            xtv[j][:, :, t * 128:(t + 1) * 128],
                            wvv[j][:, :, g * 512:(g + 1) * 512],
                            start=(j == 0), stop=(j == 3), perf_mode=DR)
                    dst = v65[t][:].rearrange(
                        "p (h e) -> p h e", e=65)[:, 8 * g:8 * (g + 1), 0:64]
                    nc.scalar.activation(
                        dst, ps[:].rearrange("p (h e) -> p h e", e=64),
                        AF.Silu, scale=INV16)

            def kq_block(m, b, psp):
                """One K (b<4) or Q (b>=4) projection block for pair m."""
                if b < 4:
                    q4 = b
                    ps = psp.tile([128, 512], F32, tag="sc",
                                  name=f"psk{m}_{q4}")
                    for j in range(4):
                        nc.tensor.matmul(
                            ps[:],
                            wkv[j][:, :, m * 128:(m + 1) * 128],
                            xtv[j][:, :, q4 * 512:(q4 + 1) * 512],
                            start=(j == 0), stop=(j == 3), perf_mode=DR)
                    nc.scalar.activation(
                        kt8[m][:, q4 * 512:(q4 + 1) * 512], ps[:],
                        AF.Silu, scale=INV16)
                else:
                    gg = b - 4
                    ps = psp.tile([128, 512], F32, tag="sc", name=f"psq{m}{gg}")
                    for j in range(4):
                        nc.tensor.matmul(
                            ps[:],
                            wqv[j][:, :, m * 128:(m + 1) * 128],
                            xtv[j][:, :, gg * 512:(gg + 1) * 512],
                            start=(j == 0), stop=(j == 3), perf_mode=DR)
                    nc.scalar.activation(
                        qtA[m][0:64, gg * 512:(gg + 1) * 512], ps[0:64, :],
                        AF.Silu, scale=INV16)
                    nc.scalar.activation(
                        qtB[m][64:128, gg * 512:(gg + 1) * 512], ps[64:128, :],
                        AF.Silu, scale=INV16)

            def proj_kq(m, psp):
                for b in range(6):
                    kq_block(m, b, psp)

            exp_idx = [0]

            def emit_exp(dst_bf16, src_psum, name, force=None):
                eng = force or EXP_PAT[exp_idx[0] % len(EXP_PAT)]
                if force is None:
                    exp_idx[0] += 1
                if eng == "A":
                    nc.scalar.activation(dst_bf16, src_psum, AF.Exp,
                                         scale=EXP_SCL)
                else:
                    i16v = dst_bf16.bitcast(I16)
                    nc.vector.tensor_scalar(
                        out=i16v, in0=src_psum, scalar1=SCH_A, scalar2=SCH_B,
                        op0=ALU.mult, op1=ALU.add)

            def attn_pair(m, ps_sc, ps_nv, nvsp, interleave=None,
                          force_eng=None):
                """Attention for head pair m: [65, LH] psum accumulation
                (v1 orientation: 512-free av matmuls), denominators in row
                64 via the v65 ones column."""
                hA, hB = 2 * m, 2 * m + 1
                nvA = ps_nv.tile([65, LH], F32, tag="nv", name=f"nvA{m}")
                nvB = ps_nv.tile([65, LH], F32, tag="nv", name=f"nvB{m}")
                pend = None
                for g in range(2):
                    for kc in range(NKC):
                        if interleave is not None:
                            interleave(g, kc)
                        scA = ps_sc.tile([128, 512], F32, tag="sc",
                                         name=f"scA{m}_{g}_{kc}")
                        scB = ps_sc.tile([128, 512], F32, tag="sc",
                                         name=f"scB{m}_{g}_{kc}")
                        nc.tensor.matmul(
                            scA[:],
                            ktb[m][:, :, kc * 128:(kc + 1) * 128],
                            qtAb[m][:, :, g * 512:(g + 1) * 512],
                            start=True, stop=True, perf_mode=DR)
                        nc.tensor.matmul(
                            scB[:],
                            ktb[m][:, :, kc * 128:(kc + 1) * 128],
                            qtBb[m][:, :, g * 512:(g + 1) * 512],
                            start=True, stop=True, perf_mode=DR)
                        atA = attp.tile([128, 512], BF16, tag="att",
                                        name=f"atA{m}_{g}_{kc}")
                        atB = attp.tile([128, 512], BF16, tag="att",
                                        name=f"atB{m}_{g}_{kc}")
                        emit_exp(atA[:], scA[:], f"eA{m}{g}{kc}", force_eng)
                        emit_exp(atB[:], scB[:], f"eB{m}{g}{kc}", force_eng)
                        if pend is not None:
                            emit_av(m, pend, nvA, nvB)
                        pend = (g, kc, atA, atB)
                emit_av(m, pend, nvA, nvB)
                return attn_tail(m, nvA, nvB, nvsp)

            def emit_av(m, p, nvA, nvB):
                g, kc, atA, atB = p
                hA, hB = 2 * m, 2 * m + 1
                nc.tensor.matmul(
                    nvA[0:65, g * 512:(g + 1) * 512],
                    v65[kc][:, hA * 65:hA * 65 + 65], atA[:],
                    start=(kc == 0), stop=(kc == NKC - 1))
                nc.tensor.matmul(
                    nvB[0:65, g * 512:(g + 1) * 512],
                    v65[kc][:, hB * 65:hB * 65 + 65], atB[:],
                    start=(kc == 0), stop=(kc == NKC - 1))

            def attn_tail(m, nvA, nvB, nvsp):
                """Stage unnormalized head outputs (bf16) + denominator rows
                into dstk; normalization happens batched in norm_half."""
                hA, hB = 2 * m, 2 * m + 1
                nvs = nvsp.tile([128, LH], BF16, tag="nvs", name=f"nvs{m}")
                nc.scalar.copy(nvs[0:64, :], nvA[0:64, :])
                nc.vector.tensor_copy(nvs[64:128, :], nvB[0:64, :])
                nc.scalar.copy(
                    dstk[32 * (hA // 4):32 * (hA // 4) + 1,
                         (hA % 4) * LH:(hA % 4 + 1) * LH], nvA[64:65, :])
                nc.vector.tensor_copy(
                    dstk[32 * (hB // 4):32 * (hB // 4) + 1,
                         (hB % 4) * LH:(hB % 4 + 1) * LH], nvB[64:65, :])
                return nvs

            def norm_half(half, nvs4, ps_sc):
                """Gather denoms, batched reciprocal, selector broadcast
                (x4 folded into sel), normalize into fp8 nvu2."""
                base = 32 * half
                for i, k in enumerate((2 * half, 2 * half + 1)):
                    nc.sync.dma_start(
                        dsb[base + 4 * i:base + 4 * (i + 1), :],
                        dstk[32 * k:32 * k + 1, :].rearrange(
                            "p (b n) -> p b n", n=LH))
                with nc.allow_low_precision("bf16 softmax denominators"):
                    nc.vector.reciprocal(dsb[base:base + 8, :],
                                         dsb[base:base + 8, :])
                for j in range(4 * half, 4 * (half + 1)):
                    jl = j % 4
                    for g in range(2):
                        bc = ps_sc.tile([128, 512], F32, tag="sc",
                                        name=f"bc{j}{g}")
                        nc.tensor.matmul(
                            bc[:],
                            sel[base:base + 8, jl * 128:(jl + 1) * 128],
                            dsb[base:base + 8, g * 512:(g + 1) * 512],
                            start=True, stop=True)
                        nc.vector.tensor_tensor(
                            out=nvu2[j // 2][:, (j % 2) * LH + g * 512:
                                             (j % 2) * LH + (g + 1) * 512],
                            in0=nvs4[j][:, g * 512:(g + 1) * 512],
                            in1=bc[:], op=ALU.mult)

            tsb8 = [per.tile([128, D], BF16, tag=f"tsb{t}", name=f"tsb{t}")
                    for t in range(NQT)]
            xrs = per.tile([128, NQT], F32, tag="xrs")
            stat = per.tile([128, 8 * NQT], F32, tag="stat")

            def xr_sums():
                # per-token-tile residual sums, done early while DVE idles
                for t in range(NQT):
                    nc.vector.tensor_reduce(xrs[:, t:t + 1], xr8[t][:],
                                            axis=mybir.AxisListType.X,
                                            op=ALU.add)

            def stage3_t(t, psp):
                mp = psp.tile([128, 1024], F32, tag="sp3", name=f"mp{t}")
                for g in range(2):
                    for c2 in range(4):
                        nc.tensor.matmul(
                            mp[:, g * 512:(g + 1) * 512],
                            nvu2v[c2][:, :, t * 128:(t + 1) * 128],
                            wov[c2][:, :, g * 512:(g + 1) * 512],
                            start=(c2 == 0), stop=(c2 == 3), perf_mode=DR)
                st = stat[:].rearrange("p (t e) -> p t e", e=8)[:, t, :]
                msum = st[:, 0:1]
                msb = s3p.tile([128, D], BF16, tag="msb")
                nc.scalar.activation(msb[:], mp[:], AF.Silu, scale=1.0 / 64.0,
                                     accum_out=msum)
                tsb = tsb8[t]
                nc.vector.tensor_tensor(out=tsb[:], in0=msb[:], in1=xr8[t][:],
                                        op=ALU.add)
                sq = s3p.tile([128, D], BF16, tag="sq")
                ssq = st[:, 1:2]
                nc.scalar.activation(sq[:], tsb[:], AF.Square, accum_out=ssq)
                mean, m2e, vpe, nvh = (st[:, 2:3], st[:, 3:4], st[:, 4:5],
                                       st[:, 5:6])
                rs = st[:, 6:7]
                nc.vector.tensor_tensor(out=rs, in0=msum, in1=xrs[:, t:t + 1],
                                        op=ALU.add)
                nc.vector.tensor_scalar_mul(mean, rs, 1.0 / D)
                nc.vector.tensor_scalar(
                    out=m2e, in0=mean, scalar1=mean, scalar2=EPS,
                    op0=ALU.mult, op1=ALU.subtract)
                nc.vector.tensor_scalar(
                    out=vpe, in0=ssq, scalar1=1.0 / D, scalar2=m2e,
                    op0=ALU.mult, op1=ALU.subtract)
                nc.vector.tensor_scalar_mul(nvh, vpe, -0.5)
                yi = st[:, 7:8].bitcast(I32)
                nc.vector.tensor_scalar(
                    out=yi, in0=vpe.bitcast(I32), scalar1=1, scalar2=None,
                    op0=ALU.logical_shift_right)
                nc.vector.tensor_scalar(
                    out=yi, in0=yi, scalar1=-1, scalar2=None,
                    op0=ALU.bitwise_xor)
                nc.vector.tensor_scalar(
                    out=yi, in0=yi, scalar1=float(0x5f3759e0), scalar2=None,
                    op0=ALU.add)
                y = yi.bitcast(F32)
                yy = st[:, 6:7]           # rs is dead, reuse as scratch
                for _ in range(2):
                    nc.vector.tensor_tensor(out=yy, in0=y, in1=y, op=ALU.mult)
                    nc.vector.tensor_tensor(out=yy, in0=yy, in1=nvh,
                                            op=ALU.mult)
                    nc.vector.tensor_scalar(
                        out=yy, in0=yy, scalar1=1.0, scalar2=1.5,
                        op0=ALU.mult, op1=ALU.add)
                    nc.vector.tensor_tensor(out=y, in0=y, in1=yy, op=ALU.mult)
                osb = s3p.tile([128, D], F32, tag="osb")
                nc.gpsimd.tensor_scalar(
                    out=osb[:], in0=tsb[:], scalar1=mean, scalar2=y,
                    op0=ALU.subtract, op1=ALU.mult)
                nc.sync.dma_start(out_ext[t * 128:(t + 1) * 128, :], osb[:])

            # ------------------------- schedule -------------------------
            for _rep in range(reps):
                input_dmas()
                setup_once()
                xr_sums()
                exp_idx[0] = 0
                nvs8 = [None] * NC8
                with (
                    tc.tile_pool(name="pn", bufs=2, space="PSUM") as ps_nv,
                    tc.tile_pool(name="nvsp", bufs=5) as nvsp,
                    tc.tile_pool(name="sc", bufs=4, space="PSUM") as ps_sc,
                ):
                    proj_kq(0, ps_sc)
                    proj_kq(1, ps_sc)
                    kq_blocks = [(m, b) for m in range(2, NC8)
                                 for b in range(6)]
                    kqi = [0]

                    def ilv0(g, kc):
                        if g == 0:
                            proj_v(kc, ps_sc)
                        it = g * NKC + kc
                        take = 2 if it >= 28 else 1
                        for _ in range(take):
                            if kqi[0] < len(kq_blocks):
                                mm, bb = kq_blocks[kqi[0]]
                                kqi[0] += 1
                                kq_block(mm, bb, ps_sc)

                    nvs8[0] = attn_pair(0, ps_sc, ps_nv, nvsp,
                                        interleave=ilv0, force_eng="D")
                    while kqi[0] < len(kq_blocks):
                        mm, bb = kq_blocks[kqi[0]]
                        kqi[0] += 1
                        kq_block(mm, bb, ps_sc)
                    for m in range(1, NC8):
                        nvs8[m] = attn_pair(m, ps_sc, ps_nv, nvsp)
                        if m == 4:
                            norm_half(0, nvs8, ps_sc)
                    norm_half(1, nvs8, ps_sc)
                with tc.tile_pool(name="s3p", bufs=2, space="PSUM") as psp3:
                    for t in range(NQT):
                        stage3_t(t, psp3)

    nc.compile()
    return nc
